# revision 38
# baseline (speedup 1.0000x reference)
"""EdgeNetworkLayer Trainium2 kernel: 8-core SPMD, zero collectives.

Nodes are BIN-PACKED into 8 sets (not contiguous ranges) so that each core
owns exactly the edges targeting its node set, with every core's edge count
<= EP = 2048. Each core then runs the whole pipeline locally:

  z = relu(ef @ W1 + b1)                       (PE, fp16)
  h_w = h[src]                                 (indirect DMA gather, fp16)
  msgT[i,e] = sum_{k,j} z[e,k] h_w[e,j] W2[k, i*128+j] + (b2-fold)
    via 64 PT tiles: PT_t[(a,c), e] = z[e,4g+a] * h_w[e,32b+c] (DVE fp16 2x)
    contracted against W2 tiles on PE (fp16), b2 folded as a 65th tile
    with rhs = h_wT.
  m[n] = segment_sum via banded one-hot S matmul (fp16, S exactly 0/1)
  GRU update on the core's own nodes (fp16 matmuls, fp32 bias via scalar
  ACT, elementwise on gpsimd in fp16)

Edges run in two halves of 1024. All transposes are PE transpose-mode
(fills PE idle while DVE builds PT); scatter+GRU for node tiles fully
covered by half-0 edges interleave between the halves. Output is fp16
(cast to fp32 on host). No DRAM round-trips for messages, no collectives.
"""
import numpy as np

N, H, E, ED, MLP_HID = 8192, 128, 16384, 16, 64
NCORES = 8
P = 128
EP = 2048                 # padded edges per core (exact bin-packed max)
ET = EP // P              # 16 edge tiles
EH = EP // 2              # 1024 edges per half
ETH = ET // 2             # 8 tiles per half
KG = 16                   # k-groups of 4
NSP = 1152                # padded nodes per core (9 node tiles)
NT = NSP // P             # 9 node tiles
WBAND = 384               # scatter band width, 128-aligned base (asserted)


def _bin_pack_nodes(tgt):
    """Assign each node to one of 8 cores so per-core edge counts are
    exactly E/NCORES (greedy largest-degree-first) with balanced node
    counts. Returns (node_core[N], per-core sorted node lists)."""
    deg = np.bincount(tgt, minlength=N)
    order = np.argsort(-deg, kind="stable")
    ecnt = np.zeros(NCORES, np.int64)
    ncnt = np.zeros(NCORES, np.int64)
    node_core = np.empty(N, np.int32)
    cap = E // NCORES
    for n in order:
        d = deg[n]
        best, bkey = -1, None
        for c in range(NCORES):
            if ecnt[c] + d > cap or ncnt[c] >= NSP:
                continue
            key = (ecnt[c], ncnt[c])
            if bkey is None or key < bkey:
                best, bkey = c, key
        if best < 0:
            best = int(np.argmin(ecnt + (ncnt >= NSP) * 10**9))
        node_core[n] = best
        ecnt[best] += d
        ncnt[best] += 1
    assert ecnt.max() <= cap + deg.max(), ecnt
    nodes = [np.sort(np.where(node_core == c)[0]) for c in range(NCORES)]
    return node_core, nodes, ecnt


def _host_prep(h, edge_index, edge_features, W1, b1, W2, b2, W_ih, W_hh, b_ih, b_hh):
    f16, f32 = np.float16, np.float32
    h = np.ascontiguousarray(h, f32)
    src_all = np.asarray(edge_index[0], np.int64)
    tgt_all = np.asarray(edge_index[1], np.int64)
    ef_all = np.asarray(edge_features, f32)

    node_core, nodes, ecnt = _bin_pack_nodes(tgt_all)
    local_slot = np.empty(N, np.int64)
    for c in range(NCORES):
        local_slot[nodes[c]] = np.arange(len(nodes[c]))

    shards = []
    for c in range(NCORES):
        m = node_core[tgt_all] == c
        s, t, ef = src_all[m], tgt_all[m], ef_all[m]
        lt = local_slot[t]
        o = np.argsort(lt, kind="stable")
        s, lt, ef = s[o], lt[o], ef[o]
        ne = len(s)
        assert ne <= EP, ne
        sp = np.zeros(EP, np.int64); sp[:ne] = s
        tp = np.full(EP, -1, np.int64); tp[:ne] = lt
        efp = np.zeros((EP, ED), f32); efp[:ne] = ef
        shards.append((sp, tp, efp, ne))

    # band plan (uniform across cores), base 128-aligned so every scatter
    # matmul uses a full [off, off+128) window with off in {0, 128, 256}
    base = np.zeros(ET, np.int64)
    for ti in range(ET):
        mins = []
        for c in range(NCORES):
            seg = shards[c][1][ti * P:(ti + 1) * P]
            seg = seg[seg >= 0]
            if len(seg):
                mins.append(seg.min())
        b = min(mins) if mins else 0
        base[ti] = min((b // P) * P, NSP - WBAND)
    for c in range(NCORES):
        t = shards[c][1]
        for ti in range(ET):
            seg = t[ti * P:(ti + 1) * P]
            seg = seg[seg >= 0]
            if len(seg):
                assert seg.min() >= base[ti] and seg.max() < base[ti] + WBAND, \
                    (c, ti, seg.min(), seg.max(), base[ti])

    contrib = [[] for _ in range(NT)]
    for ti in range(ET):
        for ng in range(int(base[ti]) // P, min(NT, (int(base[ti]) + WBAND) // P)):
            contrib[ng].append(ti)

    W2r = np.asarray(W2, f32).reshape(MLP_HID, H, H)            # [k, i, j]
    W2g = W2r.reshape(KG, 4, H, 4, 32)                          # [g, a, i, b, c]
    W2p = W2g.transpose(0, 3, 1, 4, 2).reshape(64, P, H)        # [(g,b), (a,c), i]
    W2P_host = np.ascontiguousarray(W2p.transpose(1, 0, 2).astype(f16))
    W2P32_host = np.ascontiguousarray(np.asarray(b2, f32).reshape(H, H).T.astype(f16))

    W1p = np.concatenate([np.asarray(W1, f32), np.asarray(b1, f32)[None, :]], 0)
    W1p16 = np.ascontiguousarray(W1p.astype(f16))

    W_ihT = np.ascontiguousarray(np.asarray(W_ih, f32).T.astype(f16))   # [128, 384]
    W_hhT = np.ascontiguousarray(np.asarray(W_hh, f32).T.astype(f16))
    b_ih = np.asarray(b_ih, f32)
    b_hh = np.asarray(b_hh, f32)
    b_r = (b_ih[:H] + b_hh[:H]).reshape(H, 1).astype(f32)
    b_z = (b_ih[H:2 * H] + b_hh[H:2 * H]).reshape(H, 1).astype(f32)
    b_in = b_ih[2 * H:].reshape(H, 1).astype(f32)
    b_hn = b_hh[2 * H:].reshape(H, 1).astype(f32)

    h16 = np.ascontiguousarray(h.astype(f16))
    repb = np.zeros((P, 4, P), f16)
    for b in range(4):
        for a in range(4):
            for c in range(32):
                repb[32 * b + c, b, 32 * a + c] = 1.0
    selz = np.zeros((MLP_HID, KG, P), f16)
    for g in range(KG):
        for a in range(4):
            selz[4 * g + a, g, 32 * a:32 * a + 32] = 1.0

    in_maps = []
    for c in range(NCORES):
        s, t, efp, ne = shards[c]
        efT = np.concatenate([efp.T, np.ones((1, EP), f32)], 0).astype(f16)  # [17, EP]
        srcidx = np.ascontiguousarray(s.reshape(ET, P).T).astype(np.int32)   # [128, ET]
        Sband = np.zeros((ET, P, WBAND), f16)
        for ti in range(ET):
            seg = t[ti * P:(ti + 1) * P]
            for r in range(P):
                if seg[r] >= 0:
                    Sband[ti, r, seg[r] - base[ti]] = 1.0
        Sband = np.ascontiguousarray(Sband.transpose(1, 0, 2))               # [128, ET, W]
        nl = nodes[c]
        hTsg = np.zeros((H, NSP), f32)
        hTsg[:, :len(nl)] = h[nl].T
        in_maps.append(dict(
            h16=h16, efT=efT, srcidx=srcidx, Sband=Sband, W2P=W2P_host,
            W2P32=W2P32_host, W1p=W1p16, WihT=W_ihT, WhhT=W_hhT, b_r=b_r,
            b_z=b_z, b_in=b_in, b_hn=b_hn,
            ident=np.eye(P, dtype=f16), repb=repb, selz=selz,
            hTsg=np.ascontiguousarray(hTsg.astype(f16))))
    return in_maps, base, contrib, nodes


def _build_program(base, contrib):
    import concourse.bass as bass
    import concourse.bacc as bacc
    import concourse.tile as tile
    import concourse.mybir as mybir

    f32 = mybir.dt.float32
    f16 = mybir.dt.float16
    i32 = mybir.dt.int32
    AF = mybir.ActivationFunctionType
    OP = mybir.AluOpType

    nc = bacc.Bacc("TRN2", target_bir_lowering=False, debug=False,
                   num_devices=NCORES)

    h16_d = nc.dram_tensor("h16", [N, H], f16, kind="ExternalInput")
    efT_d = nc.dram_tensor("efT", [ED + 1, EP], f16, kind="ExternalInput")
    src_d = nc.dram_tensor("srcidx", [P, ET], i32, kind="ExternalInput")
    S_d = nc.dram_tensor("Sband", [P, ET, WBAND], f16, kind="ExternalInput")
    W2P_d = nc.dram_tensor("W2P", [P, 64, H], f16, kind="ExternalInput")
    W2P32_d = nc.dram_tensor("W2P32", [P, H], f16, kind="ExternalInput")
    W1p_d = nc.dram_tensor("W1p", [ED + 1, MLP_HID], f16, kind="ExternalInput")
    WihT_d = nc.dram_tensor("WihT", [H, 3 * H], f16, kind="ExternalInput")
    WhhT_d = nc.dram_tensor("WhhT", [H, 3 * H], f16, kind="ExternalInput")
    br_d = nc.dram_tensor("b_r", [H, 1], f32, kind="ExternalInput")
    bz_d = nc.dram_tensor("b_z", [H, 1], f32, kind="ExternalInput")
    bin_d = nc.dram_tensor("b_in", [H, 1], f32, kind="ExternalInput")
    bhn_d = nc.dram_tensor("b_hn", [H, 1], f32, kind="ExternalInput")
    hTsg_d = nc.dram_tensor("hTsg", [H, NSP], f16, kind="ExternalInput")
    ident_d = nc.dram_tensor("ident", [P, P], f16, kind="ExternalInput")
    repb_d = nc.dram_tensor("repb", [P, 4, P], f16, kind="ExternalInput")
    selz_d = nc.dram_tensor("selz", [MLP_HID, KG, P], f16, kind="ExternalInput")
    out_d = nc.dram_tensor("out_hT", [H, NSP], f16, kind="ExternalOutput")

    # node tiles fully covered by half-0 edge tiles run between the halves
    ngA = [ng for ng in range(NT) if contrib[ng] and max(contrib[ng]) < ETH]
    ngB = [ng for ng in range(NT) if ng not in ngA]

    with tile.TileContext(nc) as tc:
        with (
            tc.tile_pool(name="const", bufs=1) as cp,
            tc.tile_pool(name="dram", bufs=1, space="DRAM") as dram,
            tc.tile_pool(name="work", bufs=1) as wp,
        ):
            # ---------- urgent small loads on sync; gather heads gpsimd
            # one-hot helper matrices FIRST: they must cross the fabric
            # before the gather-descriptor storm begins
            selz = cp.tile([MLP_HID, KG, P], f16)
            nc.scalar.dma_start(selz[:], selz_d[:])
            repb = cp.tile([P, 4, P], f16)
            nc.scalar.dma_start(repb[:], repb_d[:])
            ident = cp.tile([P, P], f16)
            nc.scalar.dma_start(ident[:], ident_d[:])
            efT = cp.tile([ED + 1, EP], f16)
            nc.sync.dma_start(efT[:], efT_d[:])
            W1p = cp.tile([ED + 1, MLP_HID], f16)
            nc.sync.dma_start(W1p[:], W1p_d[:])
            srci = cp.tile([P, ET], i32)
            nc.sync.dma_start(srci[:], src_d[:])
            w2t32 = cp.tile([P, H], f16)
            nc.sync.dma_start(w2t32[:], W2P32_d[:])

            hw16 = wp.tile([P, ET, P], f16)
            for t in range(ET):
                # stagger the gather storm: later quarters' gathers defer so
                # small latency-critical transfers cross the fabric early
                w = (0.0, 0.012, 0.016, 0.020)[t // 4]
                with tc.tile_wait_until(w, enable=w > 0):
                    nc.gpsimd.indirect_dma_start(
                        out=hw16[:, t, :], out_offset=None, in_=h16_d[:],
                        in_offset=bass.IndirectOffsetOnAxis(ap=srci[:, t:t + 1], axis=0))

            # W2P: first groups urgent on sync, rest deferred on scalar
            W2P = cp.tile([P, 64, H], f16)
            nc.sync.dma_start(W2P[:, 0:8, :], W2P_d[:, 0:8, :])
            with tc.tile_wait_until(0.012):
                nc.scalar.dma_start(W2P[:, 8:32, :], W2P_d[:, 8:32, :])
                nc.scalar.dma_start(W2P[:, 32:64, :], W2P_d[:, 32:64, :])

            # bulk, non-urgent: deferred past the gather storm
            s_tiles = cp.tile([P, ET, WBAND], f16)
            WihT = cp.tile([H, 3 * H], f16)
            WhhT = cp.tile([H, 3 * H], f16)
            b_r = cp.tile([H, 1], f32)
            b_z = cp.tile([H, 1], f32)
            b_in = cp.tile([H, 1], f32)
            b_hn = cp.tile([H, 1], f32)
            hTsg = cp.tile([H, NSP], f16)
            with tc.tile_wait_until(0.035):
                nc.gpsimd.dma_start(s_tiles[:], S_d[:])
                nc.gpsimd.dma_start(WihT[:], WihT_d[:])
                nc.gpsimd.dma_start(WhhT[:], WhhT_d[:])
                nc.gpsimd.dma_start(b_r[:], br_d[:])
                nc.gpsimd.dma_start(b_z[:], bz_d[:])
                nc.gpsimd.dma_start(b_in[:], bin_d[:])
                nc.gpsimd.dma_start(b_hn[:], bhn_d[:])
                nc.gpsimd.dma_start(hTsg[:], hTsg_d[:])

            zT_dram = dram.tile([MLP_HID, EP], f16)

            hwT = wp.tile([P, EP], f16)
            H32 = wp.tile([P, 4, EP], f16)
            msgT16 = wp.tile([P, EP], f16)
            msg = wp.tile([P, ET, P], f16)
            mrow16 = wp.tile([P, NT, P], f16, tag="mrow")
            mT16 = wp.tile([H, NSP], f16, tag="mT")
            out_sb = wp.tile([H, NSP], f16)

            def hwt_tile(t, tp_pool, on_vec=False):
                tp = tp_pool.tile([P, P], f16, tag="tp")
                nc.tensor.transpose(tp[:], hw16[:, t, :], ident[:])
                if on_vec:
                    nc.vector.tensor_copy(hwT[:, t * P:(t + 1) * P], tp[:])
                else:
                    nc.scalar.copy(hwT[:, t * P:(t + 1) * P], tp[:])

            def h32_rep(qq, h32p):
                e0, e1 = H32BLK[qq], H32BLK[qq + 1]
                for b in range(4):
                    hp = h32p.tile([P, 512], f32, tag="h32")
                    nc.tensor.matmul(hp[:, :e1 - e0], repb[:, b, :],
                                     hwT[:, e0:e1], start=True, stop=True)
                    if qq == 0:
                        nc.vector.tensor_copy(H32[:, b, e0:e1],
                                              hp[:, :e1 - e0])
                    else:
                        nc.scalar.copy(H32[:, b, e0:e1], hp[:, :e1 - e0])

            def scatter_tile(ng, psm, tpp):
                cs = contrib[ng]
                csl = slice(ng * P, (ng + 1) * P)
                if not cs:
                    nc.gpsimd.memset(mT16[:, csl], 0.0)
                    return
                pm = psm.tile([P, H], f32, tag="pm")
                for idx, ti in enumerate(cs):
                    off = ng * P - int(base[ti])
                    assert 0 <= off <= WBAND - P, (ng, ti, off)
                    nc.tensor.matmul(
                        pm[:], s_tiles[:, ti, off:off + P], msg[:, ti, :],
                        start=(idx == 0), stop=(idx == len(cs) - 1))
                nc.scalar.copy(mrow16[:, ng, :], pm[:])
                tp = tpp.tile([P, P], f16, tag="tp")
                nc.tensor.transpose(tp[:], mrow16[:, ng, :], ident[:])
                nc.scalar.copy(mT16[:, csl], tp[:])

            def gru_chunk(c0, cw, psg, gpool, ew):
                csl = slice(c0, c0 + cw)
                rz_ps = psg.tile([H, 2, 256], f32, tag="rzp")
                gg_ps = psg.tile([H, 2, 256], f32, tag="ggp")
                gin_ps = gg_ps[:, 0, :cw]
                ghn_ps = gg_ps[:, 1, :cw]
                for q in range(2):
                    nc.tensor.matmul(rz_ps[:, q, :cw], WihT[:, q * H:(q + 1) * H],
                                     mT16[:, csl], start=True, stop=False)
                    nc.tensor.matmul(rz_ps[:, q, :cw], WhhT[:, q * H:(q + 1) * H],
                                     hTsg[:, csl], start=False, stop=True)
                nc.tensor.matmul(gin_ps, WihT[:, 2 * H:3 * H],
                                 mT16[:, csl], start=True, stop=True)
                nc.tensor.matmul(ghn_ps, WhhT[:, 2 * H:3 * H],
                                 hTsg[:, csl], start=True, stop=True)
                rz = gpool.tile([H, 2, 256], f16, tag="rz")
                nc.scalar.activation(rz[:, 0, :cw], rz_ps[:, 0, :cw],
                                     AF.Sigmoid, bias=b_r[:])
                nc.scalar.activation(rz[:, 1, :cw], rz_ps[:, 1, :cw],
                                     AF.Sigmoid, bias=b_z[:])
                ghn = gpool.tile([H, 256], f16, tag="ghn")
                nc.scalar.activation(ghn[:, :cw], ghn_ps, AF.Identity,
                                     bias=b_hn[:])
                gin = gpool.tile([H, 256], f16, tag="gin")
                nc.scalar.activation(gin[:, :cw], gin_ps, AF.Identity,
                                     bias=b_in[:])
                ew.tensor_mul(ghn[:, :cw], rz[:, 0, :cw], ghn[:, :cw])
                ew.tensor_add(ghn[:, :cw], ghn[:, :cw], gin[:, :cw])
                ng_ = gpool.tile([H, 256], f16, tag="ngt")
                nc.scalar.activation(ng_[:, :cw], ghn[:, :cw], AF.Tanh)
                dif = gpool.tile([H, 256], f16, tag="dif")
                ew.tensor_sub(dif[:, :cw], hTsg[:, csl], ng_[:, :cw])
                ew.tensor_mul(dif[:, :cw], rz[:, 1, :cw], dif[:, :cw])
                ew.tensor_add(out_sb[:, csl], ng_[:, :cw], dif[:, :cw])
                nc.sync.dma_start(out_d[:, csl], out_sb[:, csl])

            # phase Z in its own PSUM scope (4 banks, freed before main)
            with tc.tile_pool(name="psz", bufs=1, space="PSUM") as psz:
                # HAM warmup: keep PE streaming before real work arrives
                wps = psz.tile([P, P], f32, tag="warm")
                for _ in range(40):
                    nc.tensor.matmul(wps[:], ident[:], ident[:],
                                     start=True, stop=True)
                zps = psz.tile([MLP_HID, EP], f32, tag="zps")
                for sc in range(EP // 512):
                    nc.tensor.matmul(zps[:, sc * 512:(sc + 1) * 512], W1p[:],
                                     efT[:, sc * 512:(sc + 1) * 512],
                                     start=True, stop=True)
                zT = wp.tile([MLP_HID, EP], f16)
                nc.vector.tensor_scalar_max(zT[:], zps[:], 0.0)
                nc.sync.dma_start(zT_dram[:], zT[:])

            with (
                tc.tile_pool(name="tpp", bufs=1, space="PSUM") as tpp,
                tc.tile_pool(name="zrp", bufs=2, space="PSUM") as zrp,
                tc.tile_pool(name="psacc", bufs=1, space="PSUM") as psacc,
                tc.tile_pool(name="h32p", bufs=1, space="PSUM") as h32p,
                tc.tile_pool(name="psm", bufs=1, space="PSUM") as psm,
                tc.tile_pool(name="psg", bufs=1, space="PSUM") as psg,
                tc.tile_pool(name="zpool", bufs=8) as zpool,
                tc.tile_pool(name="ptpool", bufs=5) as ptpool,
                tc.tile_pool(name="gpool", bufs=2) as gpool,
            ):
                BLK = [0, 512, 1024, 1536, 2048]
                global H32BLK
                H32BLK = BLK
                QN = len(BLK) - 1
                tblk = [0] * ET   # block of each edge tile
                for t in range(ET):
                    for q in range(QN):
                        if BLK[q] <= t * P < BLK[q + 1]:
                            tblk[t] = q
                qof = [tblk[max(contrib[ng])] if contrib[ng] else 0
                       for ng in range(NT)]
                for t in range(BLK[1] // P):
                    hwt_tile(t, tpp, on_vec=True)
                h32_rep(0, h32p)
                for q in range(QN):
                    qsl = slice(BLK[q], BLK[q + 1])
                    EQ = BLK[q + 1] - BLK[q]
                    acc = psacc.tile([P, 512], f32, tag="acc")
                    gstep = 1 if q == 0 else 2
                    for g in range(0, KG, gstep):
                        if q == 0:
                            Z32 = zpool.tile([P, 512], f16, tag="z32")
                            zp = zrp.tile([P, 512], f32, tag="zrp")
                            nc.tensor.matmul(zp[:, :EQ], selz[:, g, :],
                                             zT[:, qsl], start=True, stop=True)
                            nc.scalar.copy(Z32[:, :EQ], zp[:, :EQ])
                            pt = ptpool.tile([P, 4, 512], f16, tag="pt")
                            nc.vector.tensor_tensor(
                                pt[:, :, :EQ],
                                Z32[:, :EQ].unsqueeze(1)
                                .broadcast_to((P, 4, EQ)),
                                H32[:, :, qsl], OP.mult)
                            for b_ in range(4):
                                tw = 4 * g + b_
                                nc.tensor.matmul(
                                    acc[:, :EQ], W2P[:, tw, :], pt[:, b_, :EQ],
                                    start=(tw == 0), stop=False)
                        else:
                            # pair of k-groups per DVE op
                            Z32 = zpool.tile([P, 2, 512], f16, tag="z32p")
                            for j in range(2):
                                eng = nc.sync if (g + j) % 2 == 0 else nc.scalar
                                eng.dma_start(
                                    Z32[:, j, :EQ],
                                    zT_dram[4 * (g + j):4 * (g + j) + 4, qsl]
                                    .unsqueeze(1).broadcast_to((4, 32, EQ)))
                            pt = ptpool.tile([P, 2, 4, 512], f16, tag="ptp")
                            nc.vector.tensor_tensor(
                                pt[:, :, :, :EQ],
                                Z32[:, :, :EQ].unsqueeze(2)
                                .broadcast_to((P, 2, 4, EQ)),
                                H32[:, :, qsl].unsqueeze(1)
                                .broadcast_to((P, 2, 4, EQ)),
                                OP.mult)
                            for j in range(2):
                                for b_ in range(4):
                                    tw = 4 * (g + j) + b_
                                    nc.tensor.matmul(
                                        acc[:, :EQ], W2P[:, tw, :],
                                        pt[:, j, b_, :EQ],
                                        start=(tw == 0), stop=False)
                        if q < QN - 1:
                            nx0 = BLK[q + 1] // P
                            if g == 2:
                                hwt_tile(nx0, tpp)
                                hwt_tile(nx0 + 1, tpp)
                            elif g == 4:
                                hwt_tile(nx0 + 2, tpp)
                                hwt_tile(nx0 + 3, tpp)
                            elif g == 6:
                                h32_rep(q + 1, h32p)
                    nc.tensor.matmul(acc[:, :EQ], w2t32[:], hwT[:, qsl],
                                     start=False, stop=True)
                    nc.scalar.copy(msgT16[:, qsl], acc[:, :EQ])
                    for t in range(BLK[q] // P, BLK[q + 1] // P):
                        tp = tpp.tile([P, P], f16, tag="tp")
                        nc.tensor.transpose(tp[:], msgT16[:, t * P:(t + 1) * P],
                                            ident[:])
                        nc.scalar.copy(msg[:, t, :], tp[:])
                    ngs = [ng for ng in range(NT) if qof[ng] == q]
                    for ng in ngs:
                        scatter_tile(ng, psm, tpp)
                    i0 = 0
                    ci = 0
                    while i0 < len(ngs):
                        nrun = min(2, len(ngs) - i0)
                        if q < QN - 1:
                            ew = nc.gpsimd
                        else:
                            # last chunk on DVE (idle at the tail)
                            ew = nc.vector if i0 + nrun >= len(ngs) \
                                else nc.gpsimd
                        gru_chunk(ngs[i0] * P, nrun * P, psg, gpool, ew)
                        i0 += nrun
                        ci += 1

    nc.compile()
    return nc


_CACHE = {}


def _get_program(base, contrib):
    key = (tuple(base), tuple(tuple(c) for c in contrib))
    if key not in _CACHE:
        _CACHE[key] = _build_program(base, contrib)
    return _CACHE[key]


def kernel(h, edge_index, edge_features, W1, b1, W2, b2, W_ih, W_hh, b_ih, b_hh):
    from concourse import bass_utils

    in_maps, base, contrib, nodes = _host_prep(
        h, edge_index, edge_features, W1, b1, W2, b2, W_ih, W_hh, b_ih, b_hh)
    nc = _get_program(base, contrib)
    res = bass_utils.run_bass_kernel_spmd(nc, in_maps, core_ids=list(range(NCORES)))
    out = np.empty((N, H), np.float32)
    for c in range(NCORES):
        o = res.results[c]["out_hT"].astype(np.float32).T   # [NSP, H]
        out[nodes[c]] = o[:len(nodes[c])]
    return out


# revision 39
# speedup vs baseline: 1.0484x; 1.0484x over previous
"""EdgeNetworkLayer Trainium2 kernel: 8-core SPMD, zero collectives.

Nodes are BIN-PACKED into 8 sets (not contiguous ranges) so that each core
owns exactly the edges targeting its node set, with every core's edge count
<= EP = 2048. Each core then runs the whole pipeline locally:

  z = relu(ef @ W1 + b1)                       (PE, fp16)
  h_w = h[src]                                 (indirect DMA gather, fp16)
  msgT[i,e] = sum_{k,j} z[e,k] h_w[e,j] W2[k, i*128+j] + (b2-fold)
    via 64 PT tiles: PT_t[(a,c), e] = z[e,4g+a] * h_w[e,32b+c] (DVE fp16 2x)
    contracted against W2 tiles on PE (fp16), b2 folded as a 65th tile
    with rhs = h_wT.
  m[n] = segment_sum via banded one-hot S matmul (fp16, S exactly 0/1)
  GRU update on the core's own nodes (fp16 matmuls, fp32 bias via scalar
  ACT, elementwise on gpsimd in fp16)

Edges run in two halves of 1024. All transposes are PE transpose-mode
(fills PE idle while DVE builds PT); scatter+GRU for node tiles fully
covered by half-0 edges interleave between the halves. Output is fp16
(cast to fp32 on host). No DRAM round-trips for messages, no collectives.
"""
import numpy as np

N, H, E, ED, MLP_HID = 8192, 128, 16384, 16, 64
NCORES = 8
P = 128
EP = 2048                 # padded edges per core (exact bin-packed max)
ET = EP // P              # 16 edge tiles
EH = EP // 2              # 1024 edges per half
ETH = ET // 2             # 8 tiles per half
KG = 16                   # k-groups of 4
NSP = 1152                # padded nodes per core (9 node tiles)
NT = NSP // P             # 9 node tiles
WBAND = 384               # scatter band width, 128-aligned base (asserted)


def _bin_pack_nodes(tgt):
    """Assign each node to one of 8 cores so per-core edge counts are
    exactly E/NCORES (greedy largest-degree-first) with balanced node
    counts. Returns (node_core[N], per-core sorted node lists)."""
    deg = np.bincount(tgt, minlength=N)
    order = np.argsort(-deg, kind="stable")
    ecnt = np.zeros(NCORES, np.int64)
    ncnt = np.zeros(NCORES, np.int64)
    node_core = np.empty(N, np.int32)
    cap = E // NCORES
    for n in order:
        d = deg[n]
        best, bkey = -1, None
        for c in range(NCORES):
            if ecnt[c] + d > cap or ncnt[c] >= NSP:
                continue
            key = (ecnt[c], ncnt[c])
            if bkey is None or key < bkey:
                best, bkey = c, key
        if best < 0:
            best = int(np.argmin(ecnt + (ncnt >= NSP) * 10**9))
        node_core[n] = best
        ecnt[best] += d
        ncnt[best] += 1
    assert ecnt.max() <= cap + deg.max(), ecnt
    nodes = [np.sort(np.where(node_core == c)[0]) for c in range(NCORES)]
    return node_core, nodes, ecnt


def _host_prep(h, edge_index, edge_features, W1, b1, W2, b2, W_ih, W_hh, b_ih, b_hh):
    f16, f32 = np.float16, np.float32
    h = np.ascontiguousarray(h, f32)
    src_all = np.asarray(edge_index[0], np.int64)
    tgt_all = np.asarray(edge_index[1], np.int64)
    ef_all = np.asarray(edge_features, f32)

    node_core, nodes, ecnt = _bin_pack_nodes(tgt_all)
    local_slot = np.empty(N, np.int64)
    for c in range(NCORES):
        local_slot[nodes[c]] = np.arange(len(nodes[c]))

    shards = []
    for c in range(NCORES):
        m = node_core[tgt_all] == c
        s, t, ef = src_all[m], tgt_all[m], ef_all[m]
        lt = local_slot[t]
        o = np.argsort(lt, kind="stable")
        s, lt, ef = s[o], lt[o], ef[o]
        ne = len(s)
        assert ne <= EP, ne
        sp = np.zeros(EP, np.int64); sp[:ne] = s
        tp = np.full(EP, -1, np.int64); tp[:ne] = lt
        efp = np.zeros((EP, ED), f32); efp[:ne] = ef
        shards.append((sp, tp, efp, ne))

    # band plan (uniform across cores), base 128-aligned so every scatter
    # matmul uses a full [off, off+128) window with off in {0, 128, 256}
    base = np.zeros(ET, np.int64)
    for ti in range(ET):
        mins = []
        for c in range(NCORES):
            seg = shards[c][1][ti * P:(ti + 1) * P]
            seg = seg[seg >= 0]
            if len(seg):
                mins.append(seg.min())
        b = min(mins) if mins else 0
        base[ti] = min((b // P) * P, NSP - WBAND)
    for c in range(NCORES):
        t = shards[c][1]
        for ti in range(ET):
            seg = t[ti * P:(ti + 1) * P]
            seg = seg[seg >= 0]
            if len(seg):
                assert seg.min() >= base[ti] and seg.max() < base[ti] + WBAND, \
                    (c, ti, seg.min(), seg.max(), base[ti])

    contrib = [[] for _ in range(NT)]
    for ti in range(ET):
        for ng in range(int(base[ti]) // P, min(NT, (int(base[ti]) + WBAND) // P)):
            contrib[ng].append(ti)

    W2r = np.asarray(W2, f32).reshape(MLP_HID, H, H)            # [k, i, j]
    W2g = W2r.reshape(KG, 4, H, 4, 32)                          # [g, a, i, b, c]
    W2p = W2g.transpose(0, 3, 1, 4, 2).reshape(64, P, H)        # [(g,b), (a,c), i]
    W2P_host = np.ascontiguousarray(W2p.transpose(1, 0, 2).astype(f16))
    W2P32_host = np.ascontiguousarray(np.asarray(b2, f32).reshape(H, H).T.astype(f16))

    W1p = np.concatenate([np.asarray(W1, f32), np.asarray(b1, f32)[None, :]], 0)
    W1p16 = np.ascontiguousarray(W1p.astype(f16))

    W_ihT = np.ascontiguousarray(np.asarray(W_ih, f32).T.astype(f16))   # [128, 384]
    W_hhT = np.ascontiguousarray(np.asarray(W_hh, f32).T.astype(f16))
    b_ih = np.asarray(b_ih, f32)
    b_hh = np.asarray(b_hh, f32)
    b_r = (b_ih[:H] + b_hh[:H]).reshape(H, 1).astype(f32)
    b_z = (b_ih[H:2 * H] + b_hh[H:2 * H]).reshape(H, 1).astype(f32)
    b_in = b_ih[2 * H:].reshape(H, 1).astype(f32)
    b_hn = b_hh[2 * H:].reshape(H, 1).astype(f32)

    h16 = np.ascontiguousarray(h.astype(f16))
    repb = np.zeros((P, 4, P), f16)
    for b in range(4):
        for a in range(4):
            for c in range(32):
                repb[32 * b + c, b, 32 * a + c] = 1.0
    selz = np.zeros((MLP_HID, KG, P), f16)
    for g in range(KG):
        for a in range(4):
            selz[4 * g + a, g, 32 * a:32 * a + 32] = 1.0

    in_maps = []
    for c in range(NCORES):
        s, t, efp, ne = shards[c]
        efT = np.concatenate([efp.T, np.ones((1, EP), f32)], 0).astype(f16)  # [17, EP]
        srcidx = np.ascontiguousarray(s.reshape(ET, P).T).astype(np.int32)   # [128, ET]
        Sband = np.zeros((ET, P, WBAND), f16)
        for ti in range(ET):
            seg = t[ti * P:(ti + 1) * P]
            for r in range(P):
                if seg[r] >= 0:
                    Sband[ti, r, seg[r] - base[ti]] = 1.0
        Sband = np.ascontiguousarray(Sband.transpose(1, 0, 2))               # [128, ET, W]
        nl = nodes[c]
        hTsg = np.zeros((H, NSP), f32)
        hTsg[:, :len(nl)] = h[nl].T
        in_maps.append(dict(
            h16=h16, efT=efT, srcidx=srcidx, Sband=Sband, W2P=W2P_host,
            W2P32=W2P32_host, W1p=W1p16, WihT=W_ihT, WhhT=W_hhT, b_r=b_r,
            b_z=b_z, b_in=b_in, b_hn=b_hn,
            ident=np.eye(P, dtype=f16), repb=repb, selz=selz,
            hTsg=np.ascontiguousarray(hTsg.astype(f16))))
    return in_maps, base, contrib, nodes


def _build_program(base, contrib):
    import concourse.bass as bass
    import concourse.bacc as bacc
    import concourse.tile as tile
    import concourse.mybir as mybir

    f32 = mybir.dt.float32
    f16 = mybir.dt.float16
    i32 = mybir.dt.int32
    AF = mybir.ActivationFunctionType
    OP = mybir.AluOpType

    nc = bacc.Bacc("TRN2", target_bir_lowering=False, debug=False,
                   num_devices=NCORES)

    h16_d = nc.dram_tensor("h16", [N, H], f16, kind="ExternalInput")
    efT_d = nc.dram_tensor("efT", [ED + 1, EP], f16, kind="ExternalInput")
    src_d = nc.dram_tensor("srcidx", [P, ET], i32, kind="ExternalInput")
    S_d = nc.dram_tensor("Sband", [P, ET, WBAND], f16, kind="ExternalInput")
    W2P_d = nc.dram_tensor("W2P", [P, 64, H], f16, kind="ExternalInput")
    W2P32_d = nc.dram_tensor("W2P32", [P, H], f16, kind="ExternalInput")
    W1p_d = nc.dram_tensor("W1p", [ED + 1, MLP_HID], f16, kind="ExternalInput")
    WihT_d = nc.dram_tensor("WihT", [H, 3 * H], f16, kind="ExternalInput")
    WhhT_d = nc.dram_tensor("WhhT", [H, 3 * H], f16, kind="ExternalInput")
    br_d = nc.dram_tensor("b_r", [H, 1], f32, kind="ExternalInput")
    bz_d = nc.dram_tensor("b_z", [H, 1], f32, kind="ExternalInput")
    bin_d = nc.dram_tensor("b_in", [H, 1], f32, kind="ExternalInput")
    bhn_d = nc.dram_tensor("b_hn", [H, 1], f32, kind="ExternalInput")
    hTsg_d = nc.dram_tensor("hTsg", [H, NSP], f16, kind="ExternalInput")
    ident_d = nc.dram_tensor("ident", [P, P], f16, kind="ExternalInput")
    repb_d = nc.dram_tensor("repb", [P, 4, P], f16, kind="ExternalInput")
    selz_d = nc.dram_tensor("selz", [MLP_HID, KG, P], f16, kind="ExternalInput")
    out_d = nc.dram_tensor("out_hT", [H, NSP], f16, kind="ExternalOutput")

    # node tiles fully covered by half-0 edge tiles run between the halves
    ngA = [ng for ng in range(NT) if contrib[ng] and max(contrib[ng]) < ETH]
    ngB = [ng for ng in range(NT) if ng not in ngA]

    with tile.TileContext(nc) as tc:
        with (
            tc.tile_pool(name="const", bufs=1) as cp,
            tc.tile_pool(name="dram", bufs=1, space="DRAM") as dram,
            tc.tile_pool(name="work", bufs=1) as wp,
        ):
            # ---------- urgent small loads on sync; gather heads gpsimd
            # one-hot helper matrices FIRST: they must cross the fabric
            # before the gather-descriptor storm begins
            selz = cp.tile([MLP_HID, KG, P], f16)
            nc.scalar.dma_start(selz[:], selz_d[:])
            repb = cp.tile([P, 4, P], f16)
            nc.scalar.dma_start(repb[:], repb_d[:])
            ident = cp.tile([P, P], f16)
            nc.scalar.dma_start(ident[:], ident_d[:])
            efT = cp.tile([ED + 1, EP], f16)
            nc.sync.dma_start(efT[:], efT_d[:])
            W1p = cp.tile([ED + 1, MLP_HID], f16)
            nc.sync.dma_start(W1p[:], W1p_d[:])
            srci = cp.tile([P, ET], i32)
            nc.sync.dma_start(srci[:], src_d[:])
            w2t32 = cp.tile([P, H], f16)
            nc.sync.dma_start(w2t32[:], W2P32_d[:])

            hw16 = wp.tile([P, ET, P], f16)
            for t in range(ET):
                # stagger the gather storm: later quarters' gathers defer so
                # small latency-critical transfers cross the fabric early
                w = (0.0, 0.012, 0.016, 0.020)[t // 4]
                with tc.tile_wait_until(w, enable=w > 0):
                    nc.gpsimd.indirect_dma_start(
                        out=hw16[:, t, :], out_offset=None, in_=h16_d[:],
                        in_offset=bass.IndirectOffsetOnAxis(ap=srci[:, t:t + 1], axis=0))

            # W2P: first groups urgent on sync, rest deferred on scalar
            W2P = cp.tile([P, 64, H], f16)
            nc.sync.dma_start(W2P[:, 0:8, :], W2P_d[:, 0:8, :])
            with tc.tile_wait_until(0.012):
                nc.scalar.dma_start(W2P[:, 8:32, :], W2P_d[:, 8:32, :])
                nc.scalar.dma_start(W2P[:, 32:64, :], W2P_d[:, 32:64, :])

            # bulk, non-urgent: deferred past the gather storm
            s_tiles = cp.tile([P, ET, WBAND], f16)
            WihT = cp.tile([H, 3 * H], f16)
            WhhT = cp.tile([H, 3 * H], f16)
            b_r = cp.tile([H, 1], f32)
            b_z = cp.tile([H, 1], f32)
            b_in = cp.tile([H, 1], f32)
            b_hn = cp.tile([H, 1], f32)
            hTsg = cp.tile([H, NSP], f16)
            with tc.tile_wait_until(0.035):
                nc.gpsimd.dma_start(s_tiles[:], S_d[:])
                nc.gpsimd.dma_start(WihT[:], WihT_d[:])
                nc.gpsimd.dma_start(WhhT[:], WhhT_d[:])
                nc.gpsimd.dma_start(b_r[:], br_d[:])
                nc.gpsimd.dma_start(b_z[:], bz_d[:])
                nc.gpsimd.dma_start(b_in[:], bin_d[:])
                nc.gpsimd.dma_start(b_hn[:], bhn_d[:])
                nc.gpsimd.dma_start(hTsg[:], hTsg_d[:])

            zT_dram = dram.tile([MLP_HID, EP], f16)

            hwT = wp.tile([P, EP], f16)
            H32 = wp.tile([P, 4, EP], f16)
            msgT16 = wp.tile([P, EP], f16)
            msg = wp.tile([P, ET, P], f16)
            mrow16 = wp.tile([P, NT, P], f16, tag="mrow")
            mT16 = wp.tile([H, NSP], f16, tag="mT")
            out_sb = wp.tile([H, NSP], f16)

            def hwt_tile(t, tp_pool, on_vec=False):
                tp = tp_pool.tile([P, P], f16, tag="tp")
                nc.tensor.transpose(tp[:], hw16[:, t, :], ident[:])
                if on_vec:
                    nc.vector.tensor_copy(hwT[:, t * P:(t + 1) * P], tp[:])
                else:
                    nc.scalar.copy(hwT[:, t * P:(t + 1) * P], tp[:])

            def h32_rep(qq, h32p):
                e0, e1 = H32BLK[qq], H32BLK[qq + 1]
                for b in range(4):
                    hp = h32p.tile([P, 512], f32, tag="h32")
                    nc.tensor.matmul(hp[:, :e1 - e0], repb[:, b, :],
                                     hwT[:, e0:e1], start=True, stop=True)
                    if qq == 0:
                        nc.vector.tensor_copy(H32[:, b, e0:e1],
                                              hp[:, :e1 - e0])
                    else:
                        nc.scalar.copy(H32[:, b, e0:e1], hp[:, :e1 - e0])

            def scatter_tile(ng, psm, tpp):
                cs = contrib[ng]
                csl = slice(ng * P, (ng + 1) * P)
                if not cs:
                    nc.gpsimd.memset(mT16[:, csl], 0.0)
                    return
                pm = psm.tile([P, H], f32, tag="pm")
                for idx, ti in enumerate(cs):
                    off = ng * P - int(base[ti])
                    assert 0 <= off <= WBAND - P, (ng, ti, off)
                    nc.tensor.matmul(
                        pm[:], s_tiles[:, ti, off:off + P], msg[:, ti, :],
                        start=(idx == 0), stop=(idx == len(cs) - 1))
                nc.scalar.copy(mrow16[:, ng, :], pm[:])
                tp = tpp.tile([P, P], f16, tag="tp")
                nc.tensor.transpose(tp[:], mrow16[:, ng, :], ident[:])
                nc.scalar.copy(mT16[:, csl], tp[:])

            def gru_chunk(c0, cw, psg, gpool, ew):
                csl = slice(c0, c0 + cw)
                rz_ps = psg.tile([H, 2, 256], f32, tag="rzp")
                gg_ps = psg.tile([H, 2, 256], f32, tag="ggp")
                gin_ps = gg_ps[:, 0, :cw]
                ghn_ps = gg_ps[:, 1, :cw]
                for q in range(2):
                    nc.tensor.matmul(rz_ps[:, q, :cw], WihT[:, q * H:(q + 1) * H],
                                     mT16[:, csl], start=True, stop=False)
                    nc.tensor.matmul(rz_ps[:, q, :cw], WhhT[:, q * H:(q + 1) * H],
                                     hTsg[:, csl], start=False, stop=True)
                nc.tensor.matmul(gin_ps, WihT[:, 2 * H:3 * H],
                                 mT16[:, csl], start=True, stop=True)
                nc.tensor.matmul(ghn_ps, WhhT[:, 2 * H:3 * H],
                                 hTsg[:, csl], start=True, stop=True)
                rz = gpool.tile([H, 2, 256], f16, tag="rz")
                nc.scalar.activation(rz[:, 0, :cw], rz_ps[:, 0, :cw],
                                     AF.Sigmoid, bias=b_r[:])
                nc.scalar.activation(rz[:, 1, :cw], rz_ps[:, 1, :cw],
                                     AF.Sigmoid, bias=b_z[:])
                ghn = gpool.tile([H, 256], f16, tag="ghn")
                nc.scalar.activation(ghn[:, :cw], ghn_ps, AF.Identity,
                                     bias=b_hn[:])
                gin = gpool.tile([H, 256], f16, tag="gin")
                nc.scalar.activation(gin[:, :cw], gin_ps, AF.Identity,
                                     bias=b_in[:])
                ew.tensor_mul(ghn[:, :cw], rz[:, 0, :cw], ghn[:, :cw])
                ew.tensor_add(ghn[:, :cw], ghn[:, :cw], gin[:, :cw])
                ng_ = gpool.tile([H, 256], f16, tag="ngt")
                nc.scalar.activation(ng_[:, :cw], ghn[:, :cw], AF.Tanh)
                dif = gpool.tile([H, 256], f16, tag="dif")
                ew.tensor_sub(dif[:, :cw], hTsg[:, csl], ng_[:, :cw])
                ew.tensor_mul(dif[:, :cw], rz[:, 1, :cw], dif[:, :cw])
                ew.tensor_add(out_sb[:, csl], ng_[:, :cw], dif[:, :cw])
                nc.sync.dma_start(out_d[:, csl], out_sb[:, csl])

            # phase Z in its own PSUM scope (4 banks, freed before main)
            with tc.tile_pool(name="psz", bufs=1, space="PSUM") as psz:
                zps = psz.tile([MLP_HID, EP], f32, tag="zps")
                for sc in range(EP // 512):
                    nc.tensor.matmul(zps[:, sc * 512:(sc + 1) * 512], W1p[:],
                                     efT[:, sc * 512:(sc + 1) * 512],
                                     start=True, stop=True)
                zT = wp.tile([MLP_HID, EP], f16)
                nc.vector.tensor_scalar_max(zT[:], zps[:], 0.0)
                nc.sync.dma_start(zT_dram[:], zT[:])

            with (
                tc.tile_pool(name="tpp", bufs=1, space="PSUM") as tpp,
                tc.tile_pool(name="zrp", bufs=2, space="PSUM") as zrp,
                tc.tile_pool(name="psacc", bufs=1, space="PSUM") as psacc,
                tc.tile_pool(name="h32p", bufs=1, space="PSUM") as h32p,
                tc.tile_pool(name="psm", bufs=1, space="PSUM") as psm,
                tc.tile_pool(name="psg", bufs=1, space="PSUM") as psg,
                tc.tile_pool(name="zpool", bufs=8) as zpool,
                tc.tile_pool(name="ptpool", bufs=5) as ptpool,
                tc.tile_pool(name="gpool", bufs=2) as gpool,
            ):
                BLK = [0, 512, 1024, 1536, 2048]
                global H32BLK
                H32BLK = BLK
                QN = len(BLK) - 1
                tblk = [0] * ET   # block of each edge tile
                for t in range(ET):
                    for q in range(QN):
                        if BLK[q] <= t * P < BLK[q + 1]:
                            tblk[t] = q
                qof = [tblk[max(contrib[ng])] if contrib[ng] else 0
                       for ng in range(NT)]
                for t in range(BLK[1] // P):
                    hwt_tile(t, tpp, on_vec=True)
                h32_rep(0, h32p)
                for q in range(QN):
                    qsl = slice(BLK[q], BLK[q + 1])
                    EQ = BLK[q + 1] - BLK[q]
                    acc = psacc.tile([P, 512], f32, tag="acc")
                    gstep = 1 if q == 0 else 2
                    for g in range(0, KG, gstep):
                        if q == 0:
                            Z32 = zpool.tile([P, 512], f16, tag="z32")
                            zp = zrp.tile([P, 512], f32, tag="zrp")
                            nc.tensor.matmul(zp[:, :EQ], selz[:, g, :],
                                             zT[:, qsl], start=True, stop=True)
                            nc.scalar.copy(Z32[:, :EQ], zp[:, :EQ])
                            pt = ptpool.tile([P, 4, 512], f16, tag="pt")
                            nc.vector.tensor_tensor(
                                pt[:, :, :EQ],
                                Z32[:, :EQ].unsqueeze(1)
                                .broadcast_to((P, 4, EQ)),
                                H32[:, :, qsl], OP.mult)
                            for b_ in range(4):
                                tw = 4 * g + b_
                                nc.tensor.matmul(
                                    acc[:, :EQ], W2P[:, tw, :], pt[:, b_, :EQ],
                                    start=(tw == 0), stop=False)
                        else:
                            # pair of k-groups per DVE op
                            Z32 = zpool.tile([P, 2, 512], f16, tag="z32p")
                            for j in range(2):
                                eng = nc.sync if (g + j) % 2 == 0 else nc.scalar
                                eng.dma_start(
                                    Z32[:, j, :EQ],
                                    zT_dram[4 * (g + j):4 * (g + j) + 4, qsl]
                                    .unsqueeze(1).broadcast_to((4, 32, EQ)))
                            pt = ptpool.tile([P, 2, 4, 512], f16, tag="ptp")
                            nc.vector.tensor_tensor(
                                pt[:, :, :, :EQ],
                                Z32[:, :, :EQ].unsqueeze(2)
                                .broadcast_to((P, 2, 4, EQ)),
                                H32[:, :, qsl].unsqueeze(1)
                                .broadcast_to((P, 2, 4, EQ)),
                                OP.mult)
                            for j in range(2):
                                for b_ in range(4):
                                    tw = 4 * (g + j) + b_
                                    nc.tensor.matmul(
                                        acc[:, :EQ], W2P[:, tw, :],
                                        pt[:, j, b_, :EQ],
                                        start=(tw == 0), stop=False)
                        if q < QN - 1:
                            nx0 = BLK[q + 1] // P
                            if g == 2:
                                hwt_tile(nx0, tpp)
                                hwt_tile(nx0 + 1, tpp)
                            elif g == 4:
                                hwt_tile(nx0 + 2, tpp)
                                hwt_tile(nx0 + 3, tpp)
                            elif g == 6:
                                h32_rep(q + 1, h32p)
                    nc.tensor.matmul(acc[:, :EQ], w2t32[:], hwT[:, qsl],
                                     start=False, stop=True)
                    nc.scalar.copy(msgT16[:, qsl], acc[:, :EQ])
                    for t in range(BLK[q] // P, BLK[q + 1] // P):
                        tp = tpp.tile([P, P], f16, tag="tp")
                        nc.tensor.transpose(tp[:], msgT16[:, t * P:(t + 1) * P],
                                            ident[:])
                        nc.scalar.copy(msg[:, t, :], tp[:])
                    ngs = [ng for ng in range(NT) if qof[ng] == q]
                    for ng in ngs:
                        scatter_tile(ng, psm, tpp)
                    i0 = 0
                    ci = 0
                    while i0 < len(ngs):
                        nrun = min(2, len(ngs) - i0)
                        if q < QN - 1:
                            ew = nc.gpsimd
                        else:
                            # last chunk on DVE (idle at the tail)
                            ew = nc.vector if i0 + nrun >= len(ngs) \
                                else nc.gpsimd
                        gru_chunk(ngs[i0] * P, nrun * P, psg, gpool, ew)
                        i0 += nrun
                        ci += 1

    nc.compile()
    return nc


_CACHE = {}


def _get_program(base, contrib):
    key = (tuple(base), tuple(tuple(c) for c in contrib))
    if key not in _CACHE:
        _CACHE[key] = _build_program(base, contrib)
    return _CACHE[key]


def kernel(h, edge_index, edge_features, W1, b1, W2, b2, W_ih, W_hh, b_ih, b_hh):
    from concourse import bass_utils

    in_maps, base, contrib, nodes = _host_prep(
        h, edge_index, edge_features, W1, b1, W2, b2, W_ih, W_hh, b_ih, b_hh)
    nc = _get_program(base, contrib)
    res = bass_utils.run_bass_kernel_spmd(nc, in_maps, core_ids=list(range(NCORES)))
    out = np.empty((N, H), np.float32)
    for c in range(NCORES):
        o = res.results[c]["out_hT"].astype(np.float32).T   # [NSP, H]
        out[nodes[c]] = o[:len(nodes[c])]
    return out
